# revision 1
# baseline (speedup 1.0000x reference)
"""Trainium2 Bass kernel for nn_AdaptiveDecoder (shared MLP + hard-routed type heads).

Strategy:
  * Host: sort nodes by type; pad each type's count to a multiple of 8*128 and
    split evenly over 8 cores -> every core sees the SAME static layout of
    type-pure 128-row tiles, so the compiled SPMD program bakes in the
    tile->head mapping and the device does zero routing work.
  * Device: keep activations transposed ([feature, nodes]) the whole way so
    the three matmul stages chain without transposes.  LayerNorm's gamma/beta
    are folded into the head weights on the host; the per-node mean/std terms
    enter via K=1 rank-1 accumulation matmuls and one K=1 broadcast matmul.
  * Matmuls run as float32r (full-rate fp32 path on TRN2 TensorE).
"""

import sys

sys.path.insert(0, "/opt/trn_rl_repo")

from contextlib import ExitStack

import numpy as np

N_CORES = 8
LATENT, HIDDEN, OUT, TYPES = 512, 1024, 256, 3
P = 128
NB = 512  # node columns per block (moving-dim max for 4-byte matmul)
KL = LATENT // P  # 4 k-tiles, stage 1
KH = HIDDEN // P  # 8 k-tiles, stage 2 / head
MH = HIDDEN // P  # 8 m-chunks of hidden
MO = OUT // P  # 2 m-chunks of head output
LN_EPS = 1e-5


def build_program(blocks, R, use_c1=True, mm_bf16=False):
    """blocks: list of (type_idx, col_offset, n_cols); R: node columns per core."""
    import concourse.mybir as mybir
    import concourse.tile as tile
    from concourse import bacc, bass_isa

    dt = mybir.dt
    f32, f32r, bf16 = dt.float32, dt.float32r, dt.bfloat16
    mmdt = bf16 if mm_bf16 else f32r  # main matmul datapath dtype
    AF = mybir.ActivationFunctionType

    nc = bacc.Bacc("TRN2", target_bir_lowering=False, debug=False, num_devices=N_CORES)

    xt = nc.dram_tensor("xt", [LATENT, R], mmdt, kind="ExternalInput").ap()
    w1d = nc.dram_tensor("w1", [LATENT, HIDDEN], mmdt, kind="ExternalInput").ap()
    w2d = nc.dram_tensor("w2", [HIDDEN, HIDDEN], mmdt, kind="ExternalInput").ap()
    b1d = nc.dram_tensor("b1r", [P, MH], f32, kind="ExternalInput").ap()
    b2d = nc.dram_tensor("b2r", [P, MH], f32, kind="ExternalInput").ap()
    whpd = nc.dram_tensor("whp", [TYPES, HIDDEN, OUT], mmdt, kind="ExternalInput").ap()
    c1d = nc.dram_tensor("c1", [TYPES, 1, OUT], mmdt, kind="ExternalInput").ap()
    c2d = nc.dram_tensor("c2", [TYPES, 1, OUT], mmdt, kind="ExternalInput").ap()
    orowd = nc.dram_tensor("orow", [1, P], f32r, kind="ExternalInput").ap()
    outd = nc.dram_tensor("out", [OUT, R], f32, kind="ExternalOutput").ap()

    def cv(ap):  # engine-facing view of an mm-dtype tile
        return ap if mm_bf16 else ap.bitcast(f32)

    with tile.TileContext(nc) as tc, ExitStack() as ctx:
        consts = ctx.enter_context(tc.tile_pool(name="consts", bufs=1))
        xt_pool = ctx.enter_context(tc.tile_pool(name="xt", bufs=3))
        h1_pool = ctx.enter_context(tc.tile_pool(name="h1", bufs=2))
        h2_pool = ctx.enter_context(tc.tile_pool(name="h2", bufs=2))
        sq_pool = ctx.enter_context(tc.tile_pool(name="sq", bufs=1))
        hs_pool = ctx.enter_context(tc.tile_pool(name="hs", bufs=2))
        qs_pool = ctx.enter_context(tc.tile_pool(name="qs", bufs=2))
        rv_pool = ctx.enter_context(tc.tile_pool(name="rv", bufs=2))
        ab_pool = ctx.enter_context(tc.tile_pool(name="ab", bufs=2))
        out_pool = ctx.enter_context(tc.tile_pool(name="outp", bufs=2))
        ps_mlp = ctx.enter_context(tc.tile_pool(name="ps_mlp", bufs=3, space="PSUM"))
        ps_head = ctx.enter_context(tc.tile_pool(name="ps_head", bufs=2, space="PSUM"))
        ps_stat = ctx.enter_context(tc.tile_pool(name="ps_stat", bufs=2, space="PSUM"))
        ps_bc = ctx.enter_context(tc.tile_pool(name="ps_bc", bufs=1, space="PSUM"))

        # --- DMAs round-robin over engine queues: a single queue serializes
        # ~0.65us per 128KB transfer, which was the whole startup stall ---
        dma_engines = [nc.sync, nc.scalar, nc.gpsimd]
        dma_rr = [0]

        def dma(out, in_):
            eng = dma_engines[dma_rr[0] % len(dma_engines)]
            dma_rr[0] += 1
            eng.dma_start(out=out, in_=in_)

        # --- prefetch the first blocks' inputs so the PE can start before
        # the bulk of the constant DMAs land ---
        xt_prefetch = {}

        def load_xt(c0, nb):
            xt_t = xt_pool.tile([P, KL * NB], mmdt, tag="xt")
            for k in range(KL):
                dma(
                    xt_t[:, k * NB : k * NB + nb],
                    xt[k * P : (k + 1) * P, c0 : c0 + nb],
                )
            return xt_t

        for bi in range(min(3, len(blocks))):
            _, _c0, _nb = blocks[bi]
            xt_prefetch[bi] = load_xt(_c0, _nb)

        # --- constants / weights, loaded once, ordered by first use: w1
        # m-halves, then block 0's head weights, then w2 m-halves, then the
        # remaining heads ---
        w1_sb = consts.tile([P, KL * HIDDEN], mmdt)
        for half in range(2):
            for k in range(KL):
                dma(
                    w1_sb[
                        :,
                        k * HIDDEN + half * (HIDDEN // 2) : k * HIDDEN
                        + (half + 1) * (HIDDEN // 2),
                    ],
                    w1d[k * P : (k + 1) * P,
                        half * (HIDDEN // 2) : (half + 1) * (HIDDEN // 2)],
                )
        b1_sb = consts.tile([P, MH], f32)
        nc.sync.dma_start(out=b1_sb[:], in_=b1d[:])
        whp_sb = consts.tile([P, TYPES * KH * OUT], mmdt)
        t0_first = blocks[0][0] if blocks else 0
        type_order = [t0_first] + [t for t in range(TYPES) if t != t0_first]

        def load_whp(t):
            for k in range(KH):
                dma(
                    whp_sb[:, (t * KH + k) * OUT : (t * KH + k + 1) * OUT],
                    whpd[t, k * P : (k + 1) * P, :],
                )

        w2_sb = consts.tile([P, KH * HIDDEN], mmdt)

        def load_w2_half(half):
            for k in range(KH):
                dma(
                    w2_sb[
                        :,
                        k * HIDDEN + half * (HIDDEN // 2) : k * HIDDEN
                        + (half + 1) * (HIDDEN // 2),
                    ],
                    w2d[k * P : (k + 1) * P,
                        half * (HIDDEN // 2) : (half + 1) * (HIDDEN // 2)],
                )

        load_w2_half(0)
        load_w2_half(1)
        load_whp(type_order[0])
        for t in type_order[1:]:
            load_whp(t)
        b2_sb = consts.tile([P, MH], f32)
        nc.sync.dma_start(out=b2_sb[:], in_=b2d[:])
        c1_sb = consts.tile([1, TYPES * OUT], mmdt)
        c2_sb = consts.tile([1, TYPES * OUT], mmdt)
        for t in range(TYPES):
            nc.sync.dma_start(out=c1_sb[:, t * OUT : (t + 1) * OUT], in_=c1d[t])
            nc.sync.dma_start(out=c2_sb[:, t * OUT : (t + 1) * OUT], in_=c2d[t])
        ones_col_bf = consts.tile([P, 1], bf16)
        nc.vector.memset(ones_col_bf[:], 1.0)
        ones_row = consts.tile([1, P], f32r)  # lhsT for partition broadcast
        nc.sync.dma_start(out=ones_row[:], in_=orowd[:])
        eps_ap = consts.tile([1, 1], f32)
        nc.vector.memset(eps_ap[:], LN_EPS)
        act_warm = consts.tile([1, 1], f32)
        nc.scalar.activation(act_warm[:], eps_ap[:], AF.Sqrt)

        # --- per-block pipeline (software-pipelined: the LN-dependent PE ops
        # of block b are emitted mid-block b+1 so the PE never waits on the
        # ACT/DVE stats chain and the HAM clock stays warm) ---

        def emit_tail(t, c0, nb, ph_list, negmu, sv, rsig):
            # rank-1 corrections close the head psum accumulation groups
            for mc in range(MO):
                ph = ph_list[mc]
                nc.tensor.matmul(
                    ph[:, :nb],
                    lhsT=c2_sb[:, t * OUT + mc * P : t * OUT + (mc + 1) * P],
                    rhs=negmu[:, :nb],
                    start=False,
                    stop=not use_c1,
                )
                if use_c1:
                    nc.tensor.matmul(
                        ph[:, :nb],
                        lhsT=c1_sb[:, t * OUT + mc * P : t * OUT + (mc + 1) * P],
                        rhs=sv[:, :nb],
                        start=False,
                        stop=True,
                    )
            # broadcast rsig across partitions (K=1 matmul), stash in SBUF
            ps_a = ps_bc.tile([P, NB], f32, tag="bc")
            nc.tensor.matmul(
                ps_a[:, :nb], lhsT=ones_row[:], rhs=rsig[:, :nb],
                start=True, stop=True,
            )
            a_sb = ab_pool.tile([P, NB], f32, tag="a")
            nc.scalar.activation(a_sb[:, :nb], ps_a[:, :nb], AF.Identity)
            out_sb = out_pool.tile([P, MO * NB], f32, tag="out")
            for mc in range(MO):
                nc.vector.tensor_mul(
                    out_sb[:, mc * NB : mc * NB + nb], ph_list[mc][:, :nb],
                    a_sb[:, :nb],
                )
                nc.sync.dma_start(
                    out=outd[mc * P : (mc + 1) * P, c0 : c0 + nb],
                    in_=out_sb[:, mc * NB : mc * NB + nb],
                )

        pending = []
        TAIL_DEPTH = 1
        for bi, (t, c0, nb) in enumerate(blocks):
            xt_t = xt_prefetch.pop(bi, None)
            if xt_t is None:
                xt_t = load_xt(c0, nb)

            # stage 1: h1^T = relu(W1^T x + b1)   [HIDDEN, nb]
            h1_t = h1_pool.tile([P, MH * NB], mmdt, tag="h1")
            for m in range(MH):
                ps = ps_mlp.tile([P, NB], f32, tag="ps_mlp")
                for k in range(KL):
                    nc.tensor.matmul(
                        ps[:, :nb],
                        lhsT=w1_sb[:, k * HIDDEN + m * P : k * HIDDEN + (m + 1) * P],
                        rhs=xt_t[:, k * NB : k * NB + nb],
                        start=(k == 0),
                        stop=(k == KL - 1),
                    )
                nc.vector.tensor_scalar(
                    h1_t[:, m * NB : m * NB + nb],
                    ps[:, :nb],
                    b1_sb[:, m : m + 1],
                    0.0,
                    op0=mybir.AluOpType.add,
                    op1=mybir.AluOpType.max,
                )

            # deferred LN tails of earlier blocks slot in here: their PE
            # inputs (negmu/sv/rsig) became ready while the blocks in between
            # ran, so the rank-1 matmuls below never stall the PE
            if len(pending) >= TAIL_DEPTH:
                pending.pop(0)()

            # stage 2: h2^T = W2^T h1 + b2; squares ride along per chunk
            h2_t = h2_pool.tile([P, MH * NB], mmdt, tag="h2")
            sq_t = sq_pool.tile([P, MH * NB], bf16, tag="sq")
            for m in range(MH):
                ps = ps_mlp.tile([P, NB], f32, tag="ps_mlp")
                for k in range(KH):
                    nc.tensor.matmul(
                        ps[:, :nb],
                        lhsT=w2_sb[:, k * HIDDEN + m * P : k * HIDDEN + (m + 1) * P],
                        rhs=h1_t[:, k * NB : k * NB + nb],
                        start=(k == 0),
                        stop=(k == KH - 1),
                    )
                nc.scalar.activation(
                    h2_t[:, m * NB : m * NB + nb],
                    ps[:, :nb],
                    AF.Identity,
                    bias=b2_sb[:, m : m + 1],
                )
                nc.vector.tensor_mul(
                    sq_t[:, m * NB : m * NB + nb],
                    cv(h2_t[:, m * NB : m * NB + nb]),
                    cv(h2_t[:, m * NB : m * NB + nb]),
                )

            # head main matmuls: only need h2, so they keep the PE hot while
            # the stats chain below runs on ACT/DVE
            ph_list = []
            for mc in range(MO):
                ph = ps_head.tile([P, NB], f32, tag="head")
                for k in range(KH):
                    nc.tensor.matmul(
                        ph[:, :nb],
                        lhsT=whp_sb[
                            :,
                            (t * KH + k) * OUT + mc * P : (t * KH + k) * OUT
                            + (mc + 1) * P,
                        ],
                        rhs=h2_t[:, k * NB : k * NB + nb],
                        start=(k == 0),
                        stop=False,
                    )
                ph_list.append(ph)

            # LN stats: pairwise-add tile pairs on DVE, then column sums of
            # the halved sets via ones-matmul (keeps PE work low)
            hs_t = hs_pool.tile([P, (MH // 2) * NB], bf16, tag="hs")
            qs_t = qs_pool.tile([P, (MH // 2) * NB], bf16, tag="qs")
            for k in range(MH // 2):
                nc.vector.tensor_add(
                    hs_t[:, k * NB : k * NB + nb],
                    cv(h2_t[:, 2 * k * NB : 2 * k * NB + nb]),
                    cv(h2_t[:, (2 * k + 1) * NB : (2 * k + 1) * NB + nb]),
                )
                nc.vector.tensor_add(
                    qs_t[:, k * NB : k * NB + nb],
                    sq_t[:, 2 * k * NB : 2 * k * NB + nb],
                    sq_t[:, (2 * k + 1) * NB : (2 * k + 1) * NB + nb],
                )
            for k in range(MH // 4):
                nc.vector.tensor_add(
                    hs_t[:, k * NB : k * NB + nb],
                    hs_t[:, 2 * k * NB : 2 * k * NB + nb],
                    hs_t[:, (2 * k + 1) * NB : (2 * k + 1) * NB + nb],
                )
                nc.vector.tensor_add(
                    qs_t[:, k * NB : k * NB + nb],
                    qs_t[:, 2 * k * NB : 2 * k * NB + nb],
                    qs_t[:, (2 * k + 1) * NB : (2 * k + 1) * NB + nb],
                )
            nc.vector.tensor_add(
                hs_t[:, :nb], hs_t[:, :nb], hs_t[:, NB : NB + nb]
            )
            nc.vector.tensor_add(
                qs_t[:, :nb], qs_t[:, :nb], qs_t[:, NB : NB + nb]
            )
            ps_s = ps_stat.tile([1, NB], f32, tag="stat")
            nc.tensor.matmul(
                ps_s[:, :nb], lhsT=ones_col_bf[:], rhs=hs_t[:, :nb],
                start=True, stop=True,
            )
            ps_q = ps_stat.tile([1, NB], f32, tag="stat")
            nc.tensor.matmul(
                ps_q[:, :nb], lhsT=ones_col_bf[:], rhs=qs_t[:, :nb],
                start=True, stop=True,
            )

            negmu = rv_pool.tile([1, NB], mmdt, tag="negmu")
            nc.scalar.activation(
                negmu[:, :nb], ps_s[:, :nb], AF.Identity, scale=-1.0 / HIDDEN
            )
            musq = rv_pool.tile([1, NB], f32, tag="musq")
            nc.scalar.activation(
                musq[:, :nb], ps_s[:, :nb], AF.Square, scale=1.0 / HIDDEN
            )
            varv = rv_pool.tile([1, NB], f32, tag="varv")
            nc.scalar.activation(
                varv[:, :nb], ps_q[:, :nb], AF.Identity, scale=1.0 / HIDDEN
            )
            nc.vector.tensor_sub(varv[:, :nb], varv[:, :nb], musq[:, :nb])
            svf = rv_pool.tile([1, NB], f32, tag="svf")  # sqrt(var + eps)
            nc.scalar.activation(svf[:, :nb], varv[:, :nb], AF.Sqrt, bias=eps_ap[:])
            if use_c1:
                sv = rv_pool.tile([1, NB], mmdt, tag="sv")
                nc.scalar.activation(sv[:, :nb], varv[:, :nb], AF.Sqrt, bias=eps_ap[:])
            else:
                sv = None
            rsf = rv_pool.tile([1, NB], f32, tag="rsf")
            nc.vector.reciprocal_approx_fast(rsf[:, :nb], svf[:, :nb])
            rsig = rv_pool.tile([1, NB], f32r, tag="rsig")
            nc.scalar.activation(rsig[:, :nb], rsf[:, :nb], AF.Identity)

            import functools

            pending.append(functools.partial(
                emit_tail, t, c0, nb, ph_list, negmu, sv, rsig
            ))

        for p in pending:
            p()

    nc.compile()
    return nc


def plan(node_types, pad_odd=True):
    """Host-side layout plan shared by all cores.

    Returns (blocks, R, caps, idx_by_type) where idx_by_type[t][c] is the array
    of original row indices of type t assigned to core c.
    """
    node_types = np.asarray(node_types)
    counts = np.bincount(node_types, minlength=TYPES)
    caps = []  # per-core column capacity for each type (multiple of P)
    idx_by_type = []
    order = np.argsort(node_types, kind="stable")
    starts = np.concatenate([[0], np.cumsum(counts)])
    for tt in range(TYPES):
        tiles = int(-(-counts[tt] // (N_CORES * P)))  # ceil to 128-row tiles/core
        if pad_odd and tiles % 4 == 1:
            # a lone 128-col block runs f32r at 1/4 rate - same cost as 2 cols
            tiles += 1
        cap = tiles * P
        caps.append(cap)
        idx_t = order[starts[tt] : starts[tt + 1]]
        base, rem = divmod(int(counts[tt]), N_CORES)
        parts, o = [], 0
        for c in range(N_CORES):
            n = base + (1 if c < rem else 0)
            parts.append(idx_t[o : o + n])
            o += n
        idx_by_type.append(parts)
    R = sum(caps)
    blocks = []
    off = 0
    for tt in range(TYPES):
        tiles = caps[tt] // P
        j = 0
        while j < tiles:
            nt = min(NB // P, tiles - j)
            blocks.append((tt, off + j * P, nt * P))
            j += nt
        off += caps[tt]
    return blocks, R, caps, idx_by_type


def _tf32(x):
    """Round fp32 to TF32 (10-bit mantissa, round-to-nearest-even)."""
    u = np.ascontiguousarray(x, dtype=np.float32).view(np.uint32).copy()
    lsb = (u >> np.uint32(13)) & np.uint32(1)
    u += np.uint32(0x0FFF) + lsb
    u &= np.uint32(0xFFFFE000)
    return u.view(np.float32)


def prep_inputs(node_latent, w1, b1, w2, b2, ln_gamma, ln_beta, head_w, head_b,
                caps, idx_by_type, mm_bf16=False):
    """Build the 8 per-core input maps."""
    if mm_bf16:
        import ml_dtypes

        cast = lambda a: np.asarray(a, dtype=np.float32).astype(ml_dtypes.bfloat16)
    else:
        cast = _tf32
    whp = cast(ln_gamma[:, None] * head_w)  # [T, H, OUT]
    c1 = cast(np.asarray(ln_beta @ head_w + head_b)).reshape(TYPES, 1, OUT)
    c2 = cast(np.asarray(ln_gamma @ head_w)).reshape(TYPES, 1, OUT)
    b1r = np.ascontiguousarray(b1.reshape(MH, P).T).astype(np.float32)
    b2r = np.ascontiguousarray(b2.reshape(MH, P).T).astype(np.float32)
    R = sum(caps)
    in_maps = []
    for c in range(N_CORES):
        xc = np.zeros((R, LATENT), np.float32)
        off = 0
        for tt in range(TYPES):
            idx = idx_by_type[tt][c]
            xc[off : off + len(idx)] = node_latent[idx]
            off += caps[tt]
        in_maps.append(
            {
                "xt": cast(xc.T),
                "w1": cast(w1),
                "w2": cast(w2),
                "b1r": b1r,
                "b2r": b2r,
                "whp": whp,
                "c1": c1,
                "c2": c2,
                "orow": np.ones((1, P), np.float32),
            }
        )
    return in_maps


def unpack_outputs(results, caps, idx_by_type, n_rows):
    out = np.empty((n_rows, OUT), np.float32)
    for c in range(N_CORES):
        oc = results[c]["out"]  # [OUT, R]
        off = 0
        for tt in range(TYPES):
            idx = idx_by_type[tt][c]
            out[idx] = oc[:, off : off + len(idx)].T
            off += caps[tt]
    return out


MM_BF16 = True


def kernel(node_latent, node_types, w1, b1, w2, b2, ln_gamma, ln_beta, head_w, head_b):
    from concourse.bass_utils import run_bass_kernel_spmd

    node_latent = np.asarray(node_latent, dtype=np.float32)
    node_types = np.asarray(node_types)
    blocks, R, caps, idx_by_type = plan(node_types, pad_odd=not MM_BF16)
    use_c1 = bool(np.any(np.asarray(ln_beta @ head_w + head_b)))
    nc = build_program(blocks, R, use_c1=use_c1, mm_bf16=MM_BF16)
    in_maps = prep_inputs(
        node_latent, w1, b1, w2, b2, ln_gamma, ln_beta, head_w, head_b,
        caps, idx_by_type, mm_bf16=MM_BF16,
    )
    res = run_bass_kernel_spmd(nc, in_maps, core_ids=list(range(N_CORES)))
    return unpack_outputs(res.results, caps, idx_by_type, node_latent.shape[0])



# revision 5
# speedup vs baseline: 1.0400x; 1.0400x over previous
"""Trainium2 Bass kernel for nn_AdaptiveDecoder (shared MLP + hard-routed type heads).

Strategy:
  * Host: sort nodes by type; pad each type's count to a multiple of 8*128 and
    split evenly over 8 cores -> every core sees the SAME static layout of
    type-pure 128-row tiles, so the compiled SPMD program bakes in the
    tile->head mapping and the device does zero routing work.
  * Device: keep activations transposed ([feature, nodes]) the whole way so
    the three matmul stages chain without transposes.
  * w2/b2 are mean-centered on the host (per input row, subtract the output-dim
    mean) so stage-2 output is exactly zero-mean: LayerNorm's mean path
    vanishes and variance is just sum(h^2)/H.  The variance column-sum uses an
    all-ones [128,128] lhsT so 1/sigma lands replicated on all partitions --
    no broadcast matmul needed.
  * All weights/activations packed to the device SBUF layout on the host so
    every input tensor loads with O(1) large DMAs.
"""

import sys

sys.path.insert(0, "/opt/trn_rl_repo")

from contextlib import ExitStack

import numpy as np

N_CORES = 8
LATENT, HIDDEN, OUT, TYPES = 512, 1024, 256, 3
P = 128
NB = 512  # node columns per block (psum bank limit for f32)
KL = LATENT // P  # 4 k-tiles, stage 1
KH = HIDDEN // P  # 8 k-tiles, stage 2 / head
MH = HIDDEN // P  # 8 m-chunks of hidden
MO = OUT // P  # 2 m-chunks of head output
H2 = HIDDEN // 2
LN_EPS = 1e-5


def build_program(blocks, R, use_c1=False):
    """blocks: list of (type_idx, col_offset, n_cols); R: node columns per core."""
    import concourse.mybir as mybir
    import concourse.tile as tile
    from concourse import bacc

    dt = mybir.dt
    f32, bf16 = dt.float32, dt.bfloat16
    AF = mybir.ActivationFunctionType

    nc = bacc.Bacc("TRN2", target_bir_lowering=False, debug=False, num_devices=N_CORES)

    xtd = nc.dram_tensor("xt", [P, KL, R], bf16, kind="ExternalInput").ap()
    w1d = nc.dram_tensor("w1", [P, 2, KL * H2], bf16, kind="ExternalInput").ap()
    w2d = nc.dram_tensor("w2", [P, 2, KH * H2], bf16, kind="ExternalInput").ap()
    whpd = nc.dram_tensor("whp", [P, TYPES, KH * OUT], bf16, kind="ExternalInput").ap()
    b1d = nc.dram_tensor("b1r", [P, MH], f32, kind="ExternalInput").ap()
    b2d = nc.dram_tensor("b2r", [P, MH], f32, kind="ExternalInput").ap()
    if use_c1:
        c1d = nc.dram_tensor("c1r", [P, TYPES * MO], f32, kind="ExternalInput").ap()
    outd = nc.dram_tensor("out", [P, MO, R], f32, kind="ExternalOutput").ap()

    with tile.TileContext(nc) as tc, ExitStack() as ctx:
        consts = ctx.enter_context(tc.tile_pool(name="consts", bufs=1))
        xt_pool = ctx.enter_context(tc.tile_pool(name="xt", bufs=3))
        h1_pool = ctx.enter_context(tc.tile_pool(name="h1", bufs=2))
        h2_pool = ctx.enter_context(tc.tile_pool(name="h2", bufs=2))
        sq_pool = ctx.enter_context(tc.tile_pool(name="sq", bufs=1))
        qs_pool = ctx.enter_context(tc.tile_pool(name="qs", bufs=2))
        rs_pool = ctx.enter_context(tc.tile_pool(name="rs", bufs=2))
        out_pool = ctx.enter_context(tc.tile_pool(name="outp", bufs=2))
        ps_mlp = ctx.enter_context(tc.tile_pool(name="ps_mlp", bufs=3, space="PSUM"))
        ps_head = ctx.enter_context(tc.tile_pool(name="ps_head", bufs=3, space="PSUM"))
        ps_stat = ctx.enter_context(tc.tile_pool(name="ps_stat", bufs=2, space="PSUM"))

        # steady-state DMAs round-robin over three engine queues
        dma_engines = [nc.sync, nc.scalar, nc.gpsimd]
        dma_rr = [0]

        def dma(out, in_):
            eng = dma_engines[dma_rr[0] % len(dma_engines)]
            dma_rr[0] += 1
            eng.dma_start(out=out, in_=in_)

        xt_prefetch = {}

        def load_xt(c0, nb, eng=None):
            xt_t = xt_pool.tile([P, KL, NB], bf16, tag="xt")
            if eng is None:
                dma(xt_t[:, :, :nb], xtd[:, :, c0 : c0 + nb])
            else:
                eng.dma_start(out=xt_t[:, :, :nb], in_=xtd[:, :, c0 : c0 + nb])
            return xt_t

        # --- prologue: queue assignment ordered by first use.
        # sync: xt b0 -> whp t0 -> xt b1/b2 -> whp rest
        # scalar: w1 halves;  gpsimd: biases then w2 halves
        _, _c0, _nb = blocks[0]
        xt_prefetch[0] = load_xt(_c0, _nb, eng=nc.sync)

        w1_sb = consts.tile([P, 2, KL * H2], bf16)
        for half in range(2):
            nc.scalar.dma_start(out=w1_sb[:, half, :], in_=w1d[:, half, :])
        b1_sb = consts.tile([P, MH], f32)
        nc.gpsimd.dma_start(out=b1_sb[:], in_=b1d[:])
        b2_sb = consts.tile([P, MH], f32)
        nc.gpsimd.dma_start(out=b2_sb[:], in_=b2d[:])
        w2_sb = consts.tile([P, 2, KH * H2], bf16)
        for half in range(2):
            nc.gpsimd.dma_start(out=w2_sb[:, half, :], in_=w2d[:, half, :])

        whp_sb = consts.tile([P, TYPES, KH * OUT], bf16)
        t0_first = blocks[0][0] if blocks else 0
        type_order = [t0_first] + [t for t in range(TYPES) if t != t0_first]
        nc.sync.dma_start(
            out=whp_sb[:, type_order[0], :], in_=whpd[:, type_order[0], :]
        )
        for bi in range(1, min(3, len(blocks))):
            _, _c0, _nb = blocks[bi]
            xt_prefetch[bi] = load_xt(_c0, _nb, eng=nc.sync)
        for t in type_order[1:]:
            nc.sync.dma_start(out=whp_sb[:, t, :], in_=whpd[:, t, :])
        if use_c1:
            c1_sb = consts.tile([P, TYPES * MO], f32)
            nc.sync.dma_start(out=c1_sb[:], in_=c1d[:])

        ones128 = consts.tile([P, P], bf16)
        nc.vector.memset(ones128[:], 1.0)
        eps_ap = consts.tile([P, 1], f32)
        nc.vector.memset(eps_ap[:], LN_EPS)
        act_warm = consts.tile([1, 1], f32)
        nc.scalar.activation(act_warm[:], eps_ap[:1, :], AF.Sqrt)

        # --- per-block pipeline; tail (rsig multiply + output DMA) of block
        # b-1 is emitted at the top of block b so its DVE ops never gate the
        # PE and the output DMA issues as early as possible ---

        def emit_tail(t, c0, nb, ph_list, rsig):
            out_sb = out_pool.tile([P, MO, NB], f32, tag="out")
            for mc in range(MO):
                nc.vector.tensor_mul(
                    out_sb[:, mc, :nb], ph_list[mc][:, :nb], rsig[:, :nb]
                )
                if use_c1:
                    nc.vector.tensor_scalar(
                        out_sb[:, mc, :nb],
                        out_sb[:, mc, :nb],
                        c1_sb[:, t * MO + mc : t * MO + mc + 1],
                        0.0,
                        op0=mybir.AluOpType.add,
                        op1=mybir.AluOpType.bypass,
                    )
            dma(outd[:, :, c0 : c0 + nb], out_sb[:, :, :nb])

        pending = []
        for bi, (t, c0, nb) in enumerate(blocks):
            xt_t = xt_prefetch.pop(bi, None)
            if xt_t is None:
                xt_t = load_xt(c0, nb)

            if pending:
                pending.pop(0)()

            # stage 1: h1^T = relu(W1^T x + b1)   [HIDDEN, nb]
            h1_t = h1_pool.tile([P, MH * NB], bf16, tag="h1")
            for m in range(MH):
                half, i = divmod(m, MH // 2)
                ps = ps_mlp.tile([P, NB], f32, tag="ps_mlp")
                for k in range(KL):
                    nc.tensor.matmul(
                        ps[:, :nb],
                        lhsT=w1_sb[:, half, k * H2 + i * P : k * H2 + (i + 1) * P],
                        rhs=xt_t[:, k, :nb],
                        start=(k == 0),
                        stop=(k == KL - 1),
                    )
                nc.vector.tensor_scalar(
                    h1_t[:, m * NB : m * NB + nb],
                    ps[:, :nb],
                    b1_sb[:, m : m + 1],
                    0.0,
                    op0=mybir.AluOpType.add,
                    op1=mybir.AluOpType.max,
                )

            # stage 2: h2^T = W2^T h1 + b2 (zero-mean by construction);
            # squares ride along per chunk for the variance sum
            h2_t = h2_pool.tile([P, MH * NB], bf16, tag="h2")
            sq_t = sq_pool.tile([P, MH * NB], bf16, tag="sq")
            for m in range(MH):
                half, i = divmod(m, MH // 2)
                ps = ps_mlp.tile([P, NB], f32, tag="ps_mlp")
                for k in range(KH):
                    nc.tensor.matmul(
                        ps[:, :nb],
                        lhsT=w2_sb[:, half, k * H2 + i * P : k * H2 + (i + 1) * P],
                        rhs=h1_t[:, k * NB : k * NB + nb],
                        start=(k == 0),
                        stop=(k == KH - 1),
                    )
                nc.scalar.activation(
                    h2_t[:, m * NB : m * NB + nb],
                    ps[:, :nb],
                    AF.Identity,
                    bias=b2_sb[:, m : m + 1],
                )
                nc.vector.tensor_mul(
                    sq_t[:, m * NB : m * NB + nb],
                    h2_t[:, m * NB : m * NB + nb],
                    h2_t[:, m * NB : m * NB + nb],
                )

            # head main matmuls: keep the PE hot while the stats chain runs
            ph_list = []
            for mc in range(MO):
                ph = ps_head.tile([P, NB], f32, tag="head")
                for k in range(KH):
                    nc.tensor.matmul(
                        ph[:, :nb],
                        lhsT=whp_sb[
                            :, t, k * OUT + mc * P : k * OUT + (mc + 1) * P
                        ],
                        rhs=h2_t[:, k * NB : k * NB + nb],
                        start=(k == 0),
                        stop=(k == KH - 1),
                    )
                ph_list.append(ph)

            # variance: pairwise-add squares 8->4->2->1 on DVE, column-sum via
            # ones-matmul (result replicated on all 128 partitions), then
            # sigma = sqrt(sum/H + eps) on ACT and 1/sigma on DVE
            qs_t = qs_pool.tile([P, (MH // 2) * NB], bf16, tag="qs")
            for k in range(MH // 2):
                nc.vector.tensor_add(
                    qs_t[:, k * NB : k * NB + nb],
                    sq_t[:, 2 * k * NB : 2 * k * NB + nb],
                    sq_t[:, (2 * k + 1) * NB : (2 * k + 1) * NB + nb],
                )
            for k in range(MH // 4):
                nc.vector.tensor_add(
                    qs_t[:, k * NB : k * NB + nb],
                    qs_t[:, 2 * k * NB : 2 * k * NB + nb],
                    qs_t[:, (2 * k + 1) * NB : (2 * k + 1) * NB + nb],
                )
            nc.vector.tensor_add(
                qs_t[:, :nb], qs_t[:, :nb], qs_t[:, NB : NB + nb]
            )
            ps_v = ps_stat.tile([P, NB], f32, tag="stat")
            nc.tensor.matmul(
                ps_v[:, :nb], lhsT=ones128[:], rhs=qs_t[:, :nb],
                start=True, stop=True,
            )
            sv = rs_pool.tile([P, NB], f32, tag="sv")
            nc.scalar.activation(
                sv[:, :nb], ps_v[:, :nb], AF.Sqrt,
                scale=1.0 / HIDDEN, bias=eps_ap[:],
            )
            rsig = rs_pool.tile([P, NB], f32, tag="rsig")
            nc.vector.reciprocal_approx_fast(rsig[:, :nb], sv[:, :nb])

            import functools

            pending.append(functools.partial(emit_tail, t, c0, nb, ph_list, rsig))

        for p in pending:
            p()

    nc.compile()
    return nc


def plan(node_types):
    """Host-side layout plan shared by all cores.

    Returns (blocks, R, caps, idx_by_type); the small remainder blocks are
    ordered last so the pipeline drains on minimum-width tiles.
    """
    node_types = np.asarray(node_types)
    counts = np.bincount(node_types, minlength=TYPES)
    caps = []
    idx_by_type = []
    order = np.argsort(node_types, kind="stable")
    starts = np.concatenate([[0], np.cumsum(counts)])
    for tt in range(TYPES):
        tiles = int(-(-counts[tt] // (N_CORES * P)))  # ceil to 128-row tiles/core
        caps.append(tiles * P)
        idx_t = order[starts[tt] : starts[tt + 1]]
        base, rem = divmod(int(counts[tt]), N_CORES)
        parts, o = [], 0
        for c in range(N_CORES):
            n = base + (1 if c < rem else 0)
            parts.append(idx_t[o : o + n])
            o += n
        idx_by_type.append(parts)
    R = sum(caps)
    big, small = [], []
    off = 0
    for tt in range(TYPES):
        tiles = caps[tt] // P
        j = 0
        while j < tiles:
            nt = min(NB // P, tiles - j)
            blk = (tt, off + j * P, nt * P)
            (big if nt * P == NB else small).append(blk)
            j += nt
        off += caps[tt]
    return big + small, R, caps, idx_by_type


def prep_inputs(node_latent, w1, b1, w2, b2, ln_gamma, ln_beta, head_w, head_b,
                caps, idx_by_type):
    """Build the 8 per-core input maps, packed to the device SBUF layouts."""
    import ml_dtypes

    bf16 = ml_dtypes.bfloat16

    def cast(a):
        return np.asarray(a, dtype=np.float32).astype(bf16)

    w1 = np.asarray(w1, np.float32)
    w2 = np.asarray(w2, np.float32)
    b1 = np.asarray(b1, np.float32)
    b2 = np.asarray(b2, np.float32)
    # mean-center w2/b2 over the output dim: stage-2 output becomes zero-mean
    # for every input, which LayerNorm's mean subtraction makes exact
    w2c = w2 - w2.mean(axis=1, keepdims=True)
    b2c = b2 - b2.mean()
    whp = np.asarray(ln_gamma, np.float32)[None, :, None] * np.asarray(
        head_w, np.float32
    )  # [T, H, OUT]
    c1 = (np.asarray(ln_beta, np.float32) @ np.asarray(head_w, np.float32)
          + np.asarray(head_b, np.float32))  # [T, OUT]

    w1p = cast(w1.reshape(KL, P, 2, H2).transpose(1, 2, 0, 3).reshape(P, 2, KL * H2))
    w2p = cast(w2c.reshape(KH, P, 2, H2).transpose(1, 2, 0, 3).reshape(P, 2, KH * H2))
    whpp = cast(
        whp.reshape(TYPES, KH, P, OUT).transpose(2, 0, 1, 3).reshape(P, TYPES, KH * OUT)
    )
    b1r = np.ascontiguousarray(b1.reshape(MH, P).T).astype(np.float32)
    b2r = np.ascontiguousarray(b2c.reshape(MH, P).T).astype(np.float32)
    c1r = np.ascontiguousarray(
        c1.reshape(TYPES, MO, P).transpose(2, 0, 1).reshape(P, TYPES * MO)
    ).astype(np.float32)
    use_c1 = bool(np.any(c1))

    R = sum(caps)
    in_maps = []
    for c in range(N_CORES):
        xc = np.zeros((R, LATENT), np.float32)
        off = 0
        for tt in range(TYPES):
            idx = idx_by_type[tt][c]
            xc[off : off + len(idx)] = node_latent[idx]
            off += caps[tt]
        xtp = cast(xc.T.reshape(KL, P, R).transpose(1, 0, 2))
        m = {
            "xt": xtp,
            "w1": w1p,
            "w2": w2p,
            "whp": whpp,
            "b1r": b1r,
            "b2r": b2r,
        }
        if use_c1:
            m["c1r"] = c1r
        in_maps.append(m)
    return in_maps, use_c1


def unpack_outputs(results, caps, idx_by_type, n_rows):
    out = np.empty((n_rows, OUT), np.float32)
    for c in range(N_CORES):
        oc = results[c]["out"]  # [P, MO, R]
        R = oc.shape[-1]
        flat = oc.transpose(2, 1, 0).reshape(R, OUT)  # node, (mc*P + p)
        off = 0
        for tt in range(TYPES):
            idx = idx_by_type[tt][c]
            out[idx] = flat[off : off + len(idx)]
            off += caps[tt]
    return out


def kernel(node_latent, node_types, w1, b1, w2, b2, ln_gamma, ln_beta, head_w, head_b):
    from concourse.bass_utils import run_bass_kernel_spmd

    node_latent = np.asarray(node_latent, dtype=np.float32)
    node_types = np.asarray(node_types)
    blocks, R, caps, idx_by_type = plan(node_types)
    in_maps, use_c1 = prep_inputs(
        node_latent, w1, b1, w2, b2, ln_gamma, ln_beta, head_w, head_b,
        caps, idx_by_type,
    )
    nc = build_program(blocks, R, use_c1=use_c1)
    res = run_bass_kernel_spmd(nc, in_maps, core_ids=list(range(N_CORES)))
    return unpack_outputs(res.results, caps, idx_by_type, node_latent.shape[0])


# revision 6
# speedup vs baseline: 1.0776x; 1.0362x over previous
"""Trainium2 Bass kernel for nn_AdaptiveDecoder (shared MLP + hard-routed type heads).

Strategy:
  * Host: sort nodes by type; each core gets the same static column layout:
    [t0 x 4096 | t1 x 4096 | t2 x 4096 | t0_rem x 128 | t1_rem x 128 | t2_rem x 128]
    so the compiled SPMD program bakes the tile->head mapping in and the
    device does zero routing work.  The three remainder tiles form one final
    384-column block whose head stage switches weights per 128-column segment,
    so the pipeline drains on a single small block.
  * Device: activations stay transposed ([feature, nodes]) so the three matmul
    stages chain without transposes.
  * w2/b2 are mean-centered on the host (per input row, subtract the output-dim
    mean) so stage-2 output is exactly zero-mean: LayerNorm's mean path
    vanishes and variance is just sum(h^2)/H.  The variance column-sum uses an
    all-ones [128,128] lhsT so 1/sigma lands replicated on all partitions --
    no broadcast matmul needed.
  * Stage-1 relu runs on ACT (DVE would gate psum-buffer reuse); DVE keeps the
    squares/pairwise-add/reciprocal chain and the final rsig multiply.
  * All weights packed on the host to the device SBUF layout; first-use
    ordered quarter-sized DMAs keep the startup critical path short.
"""

import sys

sys.path.insert(0, "/opt/trn_rl_repo")

from contextlib import ExitStack

import numpy as np

N_CORES = 8
LATENT, HIDDEN, OUT, TYPES = 512, 1024, 256, 3
P = 128
NB = 512  # node columns per block (psum bank limit for f32)
KL = LATENT // P  # 4 k-tiles, stage 1
KH = HIDDEN // P  # 8 k-tiles, stage 2 / head
MH = HIDDEN // P  # 8 m-chunks of hidden
MO = OUT // P  # 2 m-chunks of head output
Q1 = 256  # w1/w2 quarter width (2 m-chunks)
LN_EPS = 1e-5
FULL = (NB // P) * MH  # 32 full 128-tiles per type region (4096 cols)


def build_program(blocks, R, use_c1=False):
    """blocks: list of (types, col_offset, n_cols) where types is a list of
    (type_idx, seg_cols) head segments covering n_cols."""
    import concourse.mybir as mybir
    import concourse.tile as tile
    from concourse import bacc

    dt = mybir.dt
    f32, bf16 = dt.float32, dt.bfloat16
    AF = mybir.ActivationFunctionType

    nc = bacc.Bacc("TRN2", target_bir_lowering=False, debug=False, num_devices=N_CORES)

    xtd = nc.dram_tensor("xt", [P, KL, R], bf16, kind="ExternalInput").ap()
    w1d = nc.dram_tensor("w1", [P, MH // 2, KL * Q1], bf16, kind="ExternalInput").ap()
    w2d = nc.dram_tensor("w2", [P, MH // 2, KH * Q1], bf16, kind="ExternalInput").ap()
    whpd = nc.dram_tensor("whp", [P, TYPES, KH * OUT], bf16, kind="ExternalInput").ap()
    b1d = nc.dram_tensor("b1r", [P, MH], f32, kind="ExternalInput").ap()
    b2d = nc.dram_tensor("b2r", [P, MH], f32, kind="ExternalInput").ap()
    if use_c1:
        c1d = nc.dram_tensor("c1r", [P, TYPES * MO], f32, kind="ExternalInput").ap()
    outd = nc.dram_tensor("out", [P, MO, R], f32, kind="ExternalOutput").ap()

    with tile.TileContext(nc) as tc, ExitStack() as ctx:
        consts = ctx.enter_context(tc.tile_pool(name="consts", bufs=1))
        xt_pool = ctx.enter_context(tc.tile_pool(name="xt", bufs=3))
        h1_pool = ctx.enter_context(tc.tile_pool(name="h1", bufs=2))
        h2_pool = ctx.enter_context(tc.tile_pool(name="h2", bufs=2))
        sq_pool = ctx.enter_context(tc.tile_pool(name="sq", bufs=1))
        qs_pool = ctx.enter_context(tc.tile_pool(name="qs", bufs=2))
        rs_pool = ctx.enter_context(tc.tile_pool(name="rs", bufs=2))
        out_pool = ctx.enter_context(tc.tile_pool(name="outp", bufs=2))
        ps_mlp = ctx.enter_context(tc.tile_pool(name="ps_mlp", bufs=4, space="PSUM"))
        ps_head = ctx.enter_context(tc.tile_pool(name="ps_head", bufs=2, space="PSUM"))
        ps_stat = ctx.enter_context(tc.tile_pool(name="ps_stat", bufs=2, space="PSUM"))

        # steady-state DMAs round-robin sync/gpsimd (ACT stays compute-only)
        dma_engines = [nc.sync, nc.gpsimd]
        dma_rr = [0]

        def dma(out, in_):
            eng = dma_engines[dma_rr[0] % len(dma_engines)]
            dma_rr[0] += 1
            eng.dma_start(out=out, in_=in_)

        xt_tiles = {}

        def load_xt(bi, eng=None, split=False):
            _, c0, nb = blocks[bi]
            xt_t = xt_pool.tile([P, KL, NB], bf16, tag="xt")
            if split:  # two k-halves so the first stage-1 group starts sooner
                for ks in range(2):
                    eng.dma_start(
                        out=xt_t[:, 2 * ks : 2 * ks + 2, :nb],
                        in_=xtd[:, 2 * ks : 2 * ks + 2, c0 : c0 + nb],
                    )
            elif eng is not None:
                eng.dma_start(out=xt_t[:, :, :nb], in_=xtd[:, :, c0 : c0 + nb])
            else:
                dma(xt_t[:, :, :nb], xtd[:, :, c0 : c0 + nb])
            xt_tiles[bi] = xt_t

        # --- prologue, ordered by first use ---
        load_xt(0, eng=nc.sync, split=True)
        w1_sb = consts.tile([P, MH // 2, KL * Q1], bf16)
        for q in range(MH // 2):
            nc.scalar.dma_start(out=w1_sb[:, q, :], in_=w1d[:, q, :])
        b1_sb = consts.tile([P, MH], f32)
        nc.gpsimd.dma_start(out=b1_sb[:], in_=b1d[:])
        b2_sb = consts.tile([P, MH], f32)
        nc.gpsimd.dma_start(out=b2_sb[:], in_=b2d[:])
        w2_sb = consts.tile([P, MH // 2, KH * Q1], bf16)
        for q in range(MH // 2):
            nc.gpsimd.dma_start(out=w2_sb[:, q, :], in_=w2d[:, q, :])

        whp_sb = consts.tile([P, TYPES, KH * OUT], bf16)
        t0_first = blocks[0][0][0][0]
        type_order = [t0_first] + [t for t in range(TYPES) if t != t0_first]
        nc.sync.dma_start(
            out=whp_sb[:, type_order[0], :], in_=whpd[:, type_order[0], :]
        )
        for bi in range(1, min(3, len(blocks))):
            load_xt(bi, eng=nc.sync)
        for t in type_order[1:]:
            nc.sync.dma_start(out=whp_sb[:, t, :], in_=whpd[:, t, :])
        if use_c1:
            c1_sb = consts.tile([P, TYPES * MO], f32)
            nc.sync.dma_start(out=c1_sb[:], in_=c1d[:])

        ones128 = consts.tile([P, P], bf16)
        nc.vector.memset(ones128[:], 1.0)
        eps_ap = consts.tile([P, 1], f32)
        nc.vector.memset(eps_ap[:], LN_EPS)
        act_warm = consts.tile([1, 1], f32)
        nc.scalar.activation(act_warm[:], eps_ap[:1, :], AF.Sqrt)

        # --- per-block pipeline; tail (rsig multiply + output DMA) of block
        # b-1 is emitted at the top of block b so its DVE ops never gate the
        # PE and the output DMA issues as early as possible ---

        def emit_tail(segs, c0, nb, ph_list, rsig):
            out_sb = out_pool.tile([P, MO, NB], f32, tag="out")
            for mc in range(MO):
                nc.vector.tensor_mul(
                    out_sb[:, mc, :nb], ph_list[mc][:, :nb], rsig[:, :nb]
                )
                if use_c1:
                    s0 = 0
                    for t, sw in segs:
                        nc.vector.tensor_scalar(
                            out_sb[:, mc, s0 : s0 + sw],
                            out_sb[:, mc, s0 : s0 + sw],
                            c1_sb[:, t * MO + mc : t * MO + mc + 1],
                            0.0,
                            op0=mybir.AluOpType.add,
                            op1=mybir.AluOpType.bypass,
                        )
                        s0 += sw
            dma(outd[:, :, c0 : c0 + nb], out_sb[:, :, :nb])

        pending = []
        for bi, (segs, c0, nb) in enumerate(blocks):
            xt_t = xt_tiles.pop(bi)
            if bi + 3 < len(blocks):
                load_xt(bi + 3)

            if pending:
                pending.pop(0)()

            # stage 1: h1^T = relu(W1^T x + b1)   [HIDDEN, nb]  (relu on ACT)
            h1_t = h1_pool.tile([P, MH * NB], bf16, tag="h1")
            for m in range(MH):
                q, i = divmod(m, 2)
                ps = ps_mlp.tile([P, NB], f32, tag="ps_mlp")
                for k in range(KL):
                    nc.tensor.matmul(
                        ps[:, :nb],
                        lhsT=w1_sb[:, q, k * Q1 + i * P : k * Q1 + (i + 1) * P],
                        rhs=xt_t[:, k, :nb],
                        start=(k == 0),
                        stop=(k == KL - 1),
                    )
                nc.scalar.activation(
                    h1_t[:, m * NB : m * NB + nb],
                    ps[:, :nb],
                    AF.Relu,
                    bias=b1_sb[:, m : m + 1],
                )

            # stage 2: h2^T = W2^T h1 + b2 (zero-mean by construction);
            # squares ride along per chunk for the variance sum
            h2_t = h2_pool.tile([P, MH * NB], bf16, tag="h2")
            sq_t = sq_pool.tile([P, MH * NB], bf16, tag="sq")
            for m in range(MH):
                q, i = divmod(m, 2)
                ps = ps_mlp.tile([P, NB], f32, tag="ps_mlp")
                for k in range(KH):
                    nc.tensor.matmul(
                        ps[:, :nb],
                        lhsT=w2_sb[:, q, k * Q1 + i * P : k * Q1 + (i + 1) * P],
                        rhs=h1_t[:, k * NB : k * NB + nb],
                        start=(k == 0),
                        stop=(k == KH - 1),
                    )
                nc.scalar.activation(
                    h2_t[:, m * NB : m * NB + nb],
                    ps[:, :nb],
                    AF.Identity,
                    bias=b2_sb[:, m : m + 1],
                )
                nc.vector.tensor_mul(
                    sq_t[:, m * NB : m * NB + nb],
                    h2_t[:, m * NB : m * NB + nb],
                    h2_t[:, m * NB : m * NB + nb],
                )

            # head main matmuls: keep the PE hot while the stats chain runs.
            # Mixed blocks switch head weights per 128-column segment.
            ph_list = []
            for mc in range(MO):
                ph = ps_head.tile([P, NB], f32, tag="head")
                s0 = 0
                for t, sw in segs:
                    for k in range(KH):
                        nc.tensor.matmul(
                            ph[:, s0 : s0 + sw],
                            lhsT=whp_sb[
                                :, t, k * OUT + mc * P : k * OUT + (mc + 1) * P
                            ],
                            rhs=h2_t[:, k * NB + s0 : k * NB + s0 + sw],
                            start=(k == 0),
                            stop=(k == KH - 1),
                        )
                    s0 += sw
                ph_list.append(ph)

            # variance: pairwise-add squares 8->4->2->1 on DVE, column-sum via
            # ones-matmul (result replicated on all 128 partitions), then
            # sigma = sqrt(sum/H + eps) on ACT and 1/sigma on DVE
            qs_t = qs_pool.tile([P, (MH // 2) * NB], bf16, tag="qs")
            for k in range(MH // 2):
                nc.vector.tensor_add(
                    qs_t[:, k * NB : k * NB + nb],
                    sq_t[:, 2 * k * NB : 2 * k * NB + nb],
                    sq_t[:, (2 * k + 1) * NB : (2 * k + 1) * NB + nb],
                )
            for k in range(MH // 4):
                nc.vector.tensor_add(
                    qs_t[:, k * NB : k * NB + nb],
                    qs_t[:, 2 * k * NB : 2 * k * NB + nb],
                    qs_t[:, (2 * k + 1) * NB : (2 * k + 1) * NB + nb],
                )
            nc.vector.tensor_add(
                qs_t[:, :nb], qs_t[:, :nb], qs_t[:, NB : NB + nb]
            )
            ps_v = ps_stat.tile([P, NB], f32, tag="stat")
            nc.tensor.matmul(
                ps_v[:, :nb], lhsT=ones128[:], rhs=qs_t[:, :nb],
                start=True, stop=True,
            )
            sv = rs_pool.tile([P, NB], f32, tag="sv")
            nc.scalar.activation(
                sv[:, :nb], ps_v[:, :nb], AF.Sqrt,
                scale=1.0 / HIDDEN, bias=eps_ap[:],
            )
            rsig = rs_pool.tile([P, NB], f32, tag="rsig")
            nc.vector.reciprocal_approx_fast(rsig[:, :nb], sv[:, :nb])

            import functools

            pending.append(functools.partial(emit_tail, segs, c0, nb, ph_list, rsig))

        for p in pending:
            p()

    nc.compile()
    return nc


def plan(node_types):
    """Host-side layout plan shared by all cores.

    Column layout per core: [t x FULL*P for each type] + [t x P remainder for
    each type].  Returns (blocks, R, regions, idx_by_type) where regions[t] =
    (full_off, full_len, rem_off, rem_len) describes where type t's columns
    live, and idx_by_type[t][c] the original row indices for core c.
    """
    node_types = np.asarray(node_types)
    counts = np.bincount(node_types, minlength=TYPES)
    idx_by_type = []
    order = np.argsort(node_types, kind="stable")
    starts = np.concatenate([[0], np.cumsum(counts)])
    tiles_per_type = []
    for tt in range(TYPES):
        per_core = -(-int(counts[tt]) // N_CORES)
        tiles = -(-per_core // P)  # ceil to 128-row tiles per core
        tiles_per_type.append(tiles)
        idx_t = order[starts[tt] : starts[tt + 1]]
        base, rem = divmod(int(counts[tt]), N_CORES)
        parts, o = [], 0
        for c in range(N_CORES):
            n = base + (1 if c < rem else 0)
            parts.append(idx_t[o : o + n])
            o += n
        idx_by_type.append(parts)

    regions = []
    blocks = []
    full_off = 0
    n_full_total = sum(min(t, FULL) for t in tiles_per_type)
    rem_off = n_full_total * P
    for tt in range(TYPES):
        ftiles = min(tiles_per_type[tt], FULL)
        rtiles = tiles_per_type[tt] - ftiles
        assert rtiles <= 1, "remainder beyond one tile not supported"
        regions.append((full_off, ftiles * P, rem_off, rtiles * P))
        j = 0
        while j < ftiles:
            nt = min(NB // P, ftiles - j)
            blocks.append(([(tt, nt * P)], full_off + j * P, nt * P))
            j += nt
        full_off += ftiles * P
        rem_off += rtiles * P
    # merge per-type remainder tiles into mixed blocks of up to NB columns
    rem_segs = [
        (tt, regions[tt][3]) for tt in range(TYPES) if regions[tt][3] > 0
    ]
    while rem_segs:
        take, tot = [], 0
        while rem_segs and tot + rem_segs[0][1] <= NB:
            take.append(rem_segs.pop(0))
            tot += take[-1][1]
        blocks.append((take, regions[take[0][0]][2], tot))
    R = n_full_total * P + sum(regions[tt][3] for tt in range(TYPES))
    return blocks, R, regions, idx_by_type


def prep_inputs(node_latent, w1, b1, w2, b2, ln_gamma, ln_beta, head_w, head_b,
                regions, idx_by_type, R):
    """Build the 8 per-core input maps, packed to the device SBUF layouts."""
    import ml_dtypes

    bf16 = ml_dtypes.bfloat16

    def cast(a):
        return np.asarray(a, dtype=np.float32).astype(bf16)

    w1 = np.asarray(w1, np.float32)
    w2 = np.asarray(w2, np.float32)
    b1 = np.asarray(b1, np.float32)
    b2 = np.asarray(b2, np.float32)
    # mean-center w2/b2 over the output dim: stage-2 output becomes zero-mean
    # for every input, which LayerNorm's mean subtraction makes exact
    w2c = w2 - w2.mean(axis=1, keepdims=True)
    b2c = b2 - b2.mean()
    whp = np.asarray(ln_gamma, np.float32)[None, :, None] * np.asarray(
        head_w, np.float32
    )  # [T, H, OUT]
    c1 = (np.asarray(ln_beta, np.float32) @ np.asarray(head_w, np.float32)
          + np.asarray(head_b, np.float32))  # [T, OUT]

    # [P, MH//2, KL*Q1] quarters: w1p[p, q, k*Q1 + j] = w1[k*128+p, q*Q1 + j]
    w1p = cast(w1.reshape(KL, P, MH // 2, Q1).transpose(1, 2, 0, 3)
               .reshape(P, MH // 2, KL * Q1))
    w2p = cast(w2c.reshape(KH, P, MH // 2, Q1).transpose(1, 2, 0, 3)
               .reshape(P, MH // 2, KH * Q1))
    whpp = cast(
        whp.reshape(TYPES, KH, P, OUT).transpose(2, 0, 1, 3).reshape(P, TYPES, KH * OUT)
    )
    b1r = np.ascontiguousarray(b1.reshape(MH, P).T).astype(np.float32)
    b2r = np.ascontiguousarray(b2c.reshape(MH, P).T).astype(np.float32)
    c1r = np.ascontiguousarray(
        c1.reshape(TYPES, MO, P).transpose(2, 0, 1).reshape(P, TYPES * MO)
    ).astype(np.float32)
    use_c1 = bool(np.any(c1))

    in_maps = []
    for c in range(N_CORES):
        xc = np.zeros((R, LATENT), np.float32)
        for tt in range(TYPES):
            fo, fl, ro, rl = regions[tt]
            idx = idx_by_type[tt][c]
            nf = min(len(idx), fl)
            xc[fo : fo + nf] = node_latent[idx[:nf]]
            xc[ro : ro + len(idx) - nf] = node_latent[idx[nf:]]
        xtp = cast(xc.T.reshape(KL, P, R).transpose(1, 0, 2))
        m = {
            "xt": xtp,
            "w1": w1p,
            "w2": w2p,
            "whp": whpp,
            "b1r": b1r,
            "b2r": b2r,
        }
        if use_c1:
            m["c1r"] = c1r
        in_maps.append(m)
    return in_maps, use_c1


def unpack_outputs(results, regions, idx_by_type, n_rows):
    out = np.empty((n_rows, OUT), np.float32)
    for c in range(N_CORES):
        oc = results[c]["out"]  # [P, MO, R]
        R = oc.shape[-1]
        flat = oc.transpose(2, 1, 0).reshape(R, OUT)  # node, (mc*P + p)
        for tt in range(TYPES):
            fo, fl, ro, rl = regions[tt]
            idx = idx_by_type[tt][c]
            nf = min(len(idx), fl)
            out[idx[:nf]] = flat[fo : fo + nf]
            out[idx[nf:]] = flat[ro : ro + len(idx) - nf]
    return out


def kernel(node_latent, node_types, w1, b1, w2, b2, ln_gamma, ln_beta, head_w, head_b):
    from concourse.bass_utils import run_bass_kernel_spmd

    node_latent = np.asarray(node_latent, dtype=np.float32)
    node_types = np.asarray(node_types)
    blocks, R, regions, idx_by_type = plan(node_types)
    in_maps, use_c1 = prep_inputs(
        node_latent, w1, b1, w2, b2, ln_gamma, ln_beta, head_w, head_b,
        regions, idx_by_type, R,
    )
    nc = build_program(blocks, R, use_c1=use_c1)
    res = run_bass_kernel_spmd(nc, in_maps, core_ids=list(range(N_CORES)))
    return unpack_outputs(res.results, regions, idx_by_type, node_latent.shape[0])


# revision 9
# speedup vs baseline: 1.0884x; 1.0101x over previous
"""Trainium2 Bass kernel for nn_AdaptiveDecoder (shared MLP + hard-routed type heads).

Strategy:
  * Host: sort nodes by type; each core gets the same static column layout:
    [t0 x 4096 | t1 x 4096 | t2 x 4096 | t0_rem x 128 | t1_rem x 128 | t2_rem x 128]
    so the compiled SPMD program bakes the tile->head mapping in and the
    device does zero routing work.  The three remainder tiles form one final
    384-column block whose head stage switches weights per 128-column segment,
    so the pipeline drains on a single small block.
  * Device: activations stay transposed ([feature, nodes]) so the three matmul
    stages chain without transposes.
  * w2/b2 are mean-centered on the host (per input row, subtract the output-dim
    mean) so stage-2 output is exactly zero-mean: LayerNorm's mean path
    vanishes and variance is just sum(h^2)/H.  The variance column-sum uses an
    all-ones [128,128] lhsT so 1/sigma lands replicated on all partitions --
    no broadcast matmul needed.
  * Stage-1 relu runs on ACT (DVE would gate psum-buffer reuse); DVE keeps the
    squares/pairwise-add/reciprocal chain and the final rsig multiply.
  * All weights packed on the host to the device SBUF layout; first-use
    ordered quarter-sized DMAs keep the startup critical path short.
"""

import sys

sys.path.insert(0, "/opt/trn_rl_repo")

from contextlib import ExitStack

import numpy as np

N_CORES = 8
LATENT, HIDDEN, OUT, TYPES = 512, 1024, 256, 3
P = 128
NB = 512  # node columns per block (psum bank limit for f32)
KL = LATENT // P  # 4 k-tiles, stage 1
KH = HIDDEN // P  # 8 k-tiles, stage 2 / head
MH = HIDDEN // P  # 8 m-chunks of hidden
MO = OUT // P  # 2 m-chunks of head output
Q1 = 256  # w1/w2 quarter width (2 m-chunks)
LN_EPS = 1e-5
FULL = (NB // P) * MH  # 32 full 128-tiles per type region (4096 cols)


def build_program(blocks, R, use_c1=False):
    """blocks: list of (types, col_offset, n_cols) where types is a list of
    (type_idx, seg_cols) head segments covering n_cols."""
    import concourse.mybir as mybir
    import concourse.tile as tile
    from concourse import bacc

    dt = mybir.dt
    f32, bf16 = dt.float32, dt.bfloat16
    AF = mybir.ActivationFunctionType

    nc = bacc.Bacc("TRN2", target_bir_lowering=False, debug=False, num_devices=N_CORES)

    xtd = nc.dram_tensor("xt", [P, KL, R], bf16, kind="ExternalInput").ap()
    w1d = nc.dram_tensor("w1", [P, MH // 2, KL * Q1], bf16, kind="ExternalInput").ap()
    w2d = nc.dram_tensor("w2", [P, MH // 2, KH * Q1], bf16, kind="ExternalInput").ap()
    whpd = nc.dram_tensor("whp", [P, TYPES, KH * OUT], bf16, kind="ExternalInput").ap()
    b1d = nc.dram_tensor("b1r", [P, MH], f32, kind="ExternalInput").ap()
    b2d = nc.dram_tensor("b2r", [P, MH], f32, kind="ExternalInput").ap()
    if use_c1:
        c1d = nc.dram_tensor("c1r", [P, TYPES * MO], f32, kind="ExternalInput").ap()
    outd = nc.dram_tensor("out", [P, MO, R], f32, kind="ExternalOutput").ap()

    with tile.TileContext(nc) as tc, ExitStack() as ctx:
        consts = ctx.enter_context(tc.tile_pool(name="consts", bufs=1))
        xt_pool = ctx.enter_context(tc.tile_pool(name="xt", bufs=3))
        h1_pool = ctx.enter_context(tc.tile_pool(name="h1", bufs=2))
        h2_pool = ctx.enter_context(tc.tile_pool(name="h2", bufs=2))
        sq_pool = ctx.enter_context(tc.tile_pool(name="sq", bufs=1))
        qs_pool = ctx.enter_context(tc.tile_pool(name="qs", bufs=2))
        rs_pool = ctx.enter_context(tc.tile_pool(name="rs", bufs=2))
        out_pool = ctx.enter_context(tc.tile_pool(name="outp", bufs=2))
        ps_mlp = ctx.enter_context(tc.tile_pool(name="ps_mlp", bufs=4, space="PSUM"))
        ps_head = ctx.enter_context(tc.tile_pool(name="ps_head", bufs=2, space="PSUM"))
        ps_stat = ctx.enter_context(tc.tile_pool(name="ps_stat", bufs=2, space="PSUM"))

        # steady-state DMAs round-robin sync/gpsimd (ACT stays compute-only)
        dma_engines = [nc.sync, nc.gpsimd]
        dma_rr = [0]

        def dma(out, in_):
            eng = dma_engines[dma_rr[0] % len(dma_engines)]
            dma_rr[0] += 1
            eng.dma_start(out=out, in_=in_)

        xt_tiles = {}

        def load_xt(bi, eng=None, split=False):
            _, c0, nb = blocks[bi]
            xt_t = xt_pool.tile([P, KL, NB], bf16, tag="xt")
            if split:  # two k-halves so the first stage-1 group starts sooner
                for ks in range(2):
                    eng.dma_start(
                        out=xt_t[:, 2 * ks : 2 * ks + 2, :nb],
                        in_=xtd[:, 2 * ks : 2 * ks + 2, c0 : c0 + nb],
                    )
            elif eng is not None:
                eng.dma_start(out=xt_t[:, :, :nb], in_=xtd[:, :, c0 : c0 + nb])
            else:
                dma(xt_t[:, :, :nb], xtd[:, :, c0 : c0 + nb])
            xt_tiles[bi] = xt_t

        # --- prologue: interleave the critical-path weight quarters across
        # all four DMA queues in first-use (deadline) order ---
        w1_sb = consts.tile([P, MH // 2, KL * Q1], bf16)
        w2_sb = consts.tile([P, MH // 2, KH * Q1], bf16)
        b1_sb = consts.tile([P, MH], f32)
        b2_sb = consts.tile([P, MH], f32)
        whp_sb = consts.tile([P, TYPES, KH * OUT], bf16)
        t0_first = blocks[0][0][0][0]
        type_order = [t0_first] + [t for t in range(TYPES) if t != t0_first]

        _, _c0, _nb = blocks[0]
        xt0 = xt_pool.tile([P, KL, NB], bf16, tag="xt")
        xt_tiles[0] = xt0
        nc.sync.dma_start(out=xt0[:, 0:2, :_nb], in_=xtd[:, 0:2, _c0 : _c0 + _nb])
        nc.scalar.dma_start(out=w1_sb[:, 0, :], in_=w1d[:, 0, :])
        nc.gpsimd.dma_start(out=b1_sb[:], in_=b1d[:])
        nc.gpsimd.dma_start(out=w1_sb[:, 1, :], in_=w1d[:, 1, :])
        nc.sync.dma_start(out=xt0[:, 2:4, :_nb], in_=xtd[:, 2:4, _c0 : _c0 + _nb])
        nc.scalar.dma_start(out=w1_sb[:, 3, :], in_=w1d[:, 3, :])
        nc.sync.dma_start(out=w1_sb[:, 2, :], in_=w1d[:, 2, :])
        nc.scalar.dma_start(out=w2_sb[:, 0, :], in_=w2d[:, 0, :])
        nc.gpsimd.dma_start(out=b2_sb[:], in_=b2d[:])
        nc.gpsimd.dma_start(out=w2_sb[:, 2, :], in_=w2d[:, 2, :])
        nc.sync.dma_start(out=w2_sb[:, 1, :], in_=w2d[:, 1, :])
        nc.gpsimd.dma_start(out=w2_sb[:, 3, :], in_=w2d[:, 3, :])
        nc.scalar.dma_start(
            out=whp_sb[:, type_order[0], :], in_=whpd[:, type_order[0], :]
        )
        for bi in range(1, min(3, len(blocks))):
            load_xt(bi, eng=(nc.sync if bi % 2 else nc.gpsimd))
        for ei, t in enumerate(type_order[1:]):
            (nc.scalar if ei % 2 else nc.sync).dma_start(
                out=whp_sb[:, t, :], in_=whpd[:, t, :]
            )
        if use_c1:
            c1_sb = consts.tile([P, TYPES * MO], f32)
            nc.sync.dma_start(out=c1_sb[:], in_=c1d[:])

        ones128 = consts.tile([P, P], bf16)
        nc.vector.memset(ones128[:], 1.0)
        eps_ap = consts.tile([P, 1], f32)
        nc.vector.memset(eps_ap[:], LN_EPS)
        act_warm = consts.tile([1, 1], f32)
        nc.scalar.activation(act_warm[:], eps_ap[:1, :], AF.Sqrt)

        # --- per-block pipeline; tail (rsig multiply + output DMA) of block
        # b-1 is emitted at the top of block b so its DVE ops never gate the
        # PE and the output DMA issues as early as possible ---

        def emit_tail(segs, c0, nb, ph_list, rsig):
            out_sb = out_pool.tile([P, MO, NB], f32, tag="out")
            for mc in range(MO):
                nc.vector.tensor_mul(
                    out_sb[:, mc, :nb], ph_list[mc][:, :nb], rsig[:, :nb]
                )
                if use_c1:
                    s0 = 0
                    for t, sw in segs:
                        nc.vector.tensor_scalar(
                            out_sb[:, mc, s0 : s0 + sw],
                            out_sb[:, mc, s0 : s0 + sw],
                            c1_sb[:, t * MO + mc : t * MO + mc + 1],
                            0.0,
                            op0=mybir.AluOpType.add,
                            op1=mybir.AluOpType.bypass,
                        )
                        s0 += sw
            dma(outd[:, :, c0 : c0 + nb], out_sb[:, :, :nb])

        pending = []
        for bi, (segs, c0, nb) in enumerate(blocks):
            xt_t = xt_tiles.pop(bi)
            if bi + 3 < len(blocks):
                load_xt(bi + 3)

            if pending:
                pending.pop(0)()

            # stage 1: h1^T = relu(W1^T x + b1)   [HIDDEN, nb]  (relu on ACT)
            h1_t = h1_pool.tile([P, MH * NB], bf16, tag="h1")
            for m in range(MH):
                q, i = divmod(m, 2)
                ps = ps_mlp.tile([P, NB], f32, tag="ps_mlp")
                for k in range(KL):
                    nc.tensor.matmul(
                        ps[:, :nb],
                        lhsT=w1_sb[:, q, k * Q1 + i * P : k * Q1 + (i + 1) * P],
                        rhs=xt_t[:, k, :nb],
                        start=(k == 0),
                        stop=(k == KL - 1),
                    )
                nc.scalar.activation(
                    h1_t[:, m * NB : m * NB + nb],
                    ps[:, :nb],
                    AF.Relu,
                    bias=b1_sb[:, m : m + 1],
                )

            # stage 2: h2^T = W2^T h1 + b2 (zero-mean by construction);
            # squares ride along per chunk for the variance sum
            h2_t = h2_pool.tile([P, MH * NB], bf16, tag="h2")
            sq_t = sq_pool.tile([P, MH * NB], bf16, tag="sq")
            for m in range(MH):
                q, i = divmod(m, 2)
                ps = ps_mlp.tile([P, NB], f32, tag="ps_mlp")
                for k in range(KH):
                    nc.tensor.matmul(
                        ps[:, :nb],
                        lhsT=w2_sb[:, q, k * Q1 + i * P : k * Q1 + (i + 1) * P],
                        rhs=h1_t[:, k * NB : k * NB + nb],
                        start=(k == 0),
                        stop=(k == KH - 1),
                    )
                nc.scalar.activation(
                    h2_t[:, m * NB : m * NB + nb],
                    ps[:, :nb],
                    AF.Identity,
                    bias=b2_sb[:, m : m + 1],
                )
                nc.vector.tensor_mul(
                    sq_t[:, m * NB : m * NB + nb],
                    h2_t[:, m * NB : m * NB + nb],
                    h2_t[:, m * NB : m * NB + nb],
                )

            # head main matmuls: keep the PE hot while the stats chain runs.
            # Mixed blocks switch head weights per 128-column segment.
            ph_list = []
            for mc in range(MO):
                ph = ps_head.tile([P, NB], f32, tag="head")
                s0 = 0
                for t, sw in segs:
                    for k in range(KH):
                        nc.tensor.matmul(
                            ph[:, s0 : s0 + sw],
                            lhsT=whp_sb[
                                :, t, k * OUT + mc * P : k * OUT + (mc + 1) * P
                            ],
                            rhs=h2_t[:, k * NB + s0 : k * NB + s0 + sw],
                            start=(k == 0),
                            stop=(k == KH - 1),
                        )
                    s0 += sw
                ph_list.append(ph)

            # variance: pairwise-add squares 8->4->2->1 on DVE, column-sum via
            # ones-matmul (result replicated on all 128 partitions), then
            # sigma = sqrt(sum/H + eps) on ACT and 1/sigma on DVE
            qs_t = qs_pool.tile([P, (MH // 2) * NB], bf16, tag="qs")
            for k in range(MH // 2):
                nc.vector.tensor_add(
                    qs_t[:, k * NB : k * NB + nb],
                    sq_t[:, 2 * k * NB : 2 * k * NB + nb],
                    sq_t[:, (2 * k + 1) * NB : (2 * k + 1) * NB + nb],
                )
            for k in range(MH // 4):
                nc.vector.tensor_add(
                    qs_t[:, k * NB : k * NB + nb],
                    qs_t[:, 2 * k * NB : 2 * k * NB + nb],
                    qs_t[:, (2 * k + 1) * NB : (2 * k + 1) * NB + nb],
                )
            nc.vector.tensor_add(
                qs_t[:, :nb], qs_t[:, :nb], qs_t[:, NB : NB + nb]
            )
            ps_v = ps_stat.tile([P, NB], f32, tag="stat")
            nc.tensor.matmul(
                ps_v[:, :nb], lhsT=ones128[:], rhs=qs_t[:, :nb],
                start=True, stop=True,
            )
            sv = rs_pool.tile([P, NB], f32, tag="sv")
            nc.scalar.activation(
                sv[:, :nb], ps_v[:, :nb], AF.Sqrt,
                scale=1.0 / HIDDEN, bias=eps_ap[:],
            )
            rsig = rs_pool.tile([P, NB], f32, tag="rsig")
            nc.vector.reciprocal_approx_fast(rsig[:, :nb], sv[:, :nb])

            import functools

            pending.append(functools.partial(emit_tail, segs, c0, nb, ph_list, rsig))

        for p in pending:
            p()

    nc.compile()
    return nc


def plan(node_types):
    """Host-side layout plan shared by all cores.

    Column layout per core: [t x FULL*P for each type] + [t x P remainder for
    each type].  Returns (blocks, R, regions, idx_by_type) where regions[t] =
    (full_off, full_len, rem_off, rem_len) describes where type t's columns
    live, and idx_by_type[t][c] the original row indices for core c.
    """
    node_types = np.asarray(node_types)
    counts = np.bincount(node_types, minlength=TYPES)
    idx_by_type = []
    order = np.argsort(node_types, kind="stable")
    starts = np.concatenate([[0], np.cumsum(counts)])
    tiles_per_type = []
    for tt in range(TYPES):
        per_core = -(-int(counts[tt]) // N_CORES)
        tiles = -(-per_core // P)  # ceil to 128-row tiles per core
        tiles_per_type.append(tiles)
        idx_t = order[starts[tt] : starts[tt + 1]]
        base, rem = divmod(int(counts[tt]), N_CORES)
        parts, o = [], 0
        for c in range(N_CORES):
            n = base + (1 if c < rem else 0)
            parts.append(idx_t[o : o + n])
            o += n
        idx_by_type.append(parts)

    # full regions: whole 512-column blocks; exact-width remainders are
    # packed into one final mixed block padded up to a 128-column multiple
    per_core_max = [
        max(len(p) for p in idx_by_type[tt]) for tt in range(TYPES)
    ]
    fulls = [(m // NB) * NB for m in per_core_max]
    rems = [per_core_max[tt] - fulls[tt] for tt in range(TYPES)]
    rem_total = sum(rems)
    rem_cols = -(-rem_total // P) * P  # pad to 128-multiple
    regions = []
    blocks = []
    full_off = 0
    rem_base = sum(fulls)
    rem_off = rem_base
    for tt in range(TYPES):
        regions.append((full_off, fulls[tt], rem_off, rems[tt]))
        for j in range(fulls[tt] // NB):
            blocks.append(([(tt, NB)], full_off + j * NB, NB))
        full_off += fulls[tt]
        rem_off += rems[tt]
    if rem_cols:
        segs = [(tt, rems[tt]) for tt in range(TYPES) if rems[tt] > 0]
        segs[-1] = (segs[-1][0], segs[-1][1] + rem_cols - rem_total)
        assert rem_cols <= NB, "remainder block exceeds one NB block"
        blocks.append((segs, rem_base, rem_cols))
    R = rem_base + rem_cols
    return blocks, R, regions, idx_by_type


def prep_inputs(node_latent, w1, b1, w2, b2, ln_gamma, ln_beta, head_w, head_b,
                regions, idx_by_type, R):
    """Build the 8 per-core input maps, packed to the device SBUF layouts."""
    import ml_dtypes

    bf16 = ml_dtypes.bfloat16

    def cast(a):
        return np.asarray(a, dtype=np.float32).astype(bf16)

    w1 = np.asarray(w1, np.float32)
    w2 = np.asarray(w2, np.float32)
    b1 = np.asarray(b1, np.float32)
    b2 = np.asarray(b2, np.float32)
    # mean-center w2/b2 over the output dim: stage-2 output becomes zero-mean
    # for every input, which LayerNorm's mean subtraction makes exact
    w2c = w2 - w2.mean(axis=1, keepdims=True)
    b2c = b2 - b2.mean()
    whp = np.asarray(ln_gamma, np.float32)[None, :, None] * np.asarray(
        head_w, np.float32
    )  # [T, H, OUT]
    c1 = (np.asarray(ln_beta, np.float32) @ np.asarray(head_w, np.float32)
          + np.asarray(head_b, np.float32))  # [T, OUT]

    # [P, MH//2, KL*Q1] quarters: w1p[p, q, k*Q1 + j] = w1[k*128+p, q*Q1 + j]
    w1p = cast(w1.reshape(KL, P, MH // 2, Q1).transpose(1, 2, 0, 3)
               .reshape(P, MH // 2, KL * Q1))
    w2p = cast(w2c.reshape(KH, P, MH // 2, Q1).transpose(1, 2, 0, 3)
               .reshape(P, MH // 2, KH * Q1))
    whpp = cast(
        whp.reshape(TYPES, KH, P, OUT).transpose(2, 0, 1, 3).reshape(P, TYPES, KH * OUT)
    )
    b1r = np.ascontiguousarray(b1.reshape(MH, P).T).astype(np.float32)
    b2r = np.ascontiguousarray(b2c.reshape(MH, P).T).astype(np.float32)
    c1r = np.ascontiguousarray(
        c1.reshape(TYPES, MO, P).transpose(2, 0, 1).reshape(P, TYPES * MO)
    ).astype(np.float32)
    use_c1 = bool(np.any(c1))

    in_maps = []
    for c in range(N_CORES):
        xc = np.zeros((R, LATENT), np.float32)
        for tt in range(TYPES):
            fo, fl, ro, rl = regions[tt]
            idx = idx_by_type[tt][c]
            nf = min(len(idx), fl)
            xc[fo : fo + nf] = node_latent[idx[:nf]]
            xc[ro : ro + len(idx) - nf] = node_latent[idx[nf:]]
        xtp = cast(xc.T.reshape(KL, P, R).transpose(1, 0, 2))
        m = {
            "xt": xtp,
            "w1": w1p,
            "w2": w2p,
            "whp": whpp,
            "b1r": b1r,
            "b2r": b2r,
        }
        if use_c1:
            m["c1r"] = c1r
        in_maps.append(m)
    return in_maps, use_c1


def unpack_outputs(results, regions, idx_by_type, n_rows):
    out = np.empty((n_rows, OUT), np.float32)
    for c in range(N_CORES):
        oc = results[c]["out"]  # [P, MO, R]
        R = oc.shape[-1]
        flat = oc.transpose(2, 1, 0).reshape(R, OUT)  # node, (mc*P + p)
        for tt in range(TYPES):
            fo, fl, ro, rl = regions[tt]
            idx = idx_by_type[tt][c]
            nf = min(len(idx), fl)
            out[idx[:nf]] = flat[fo : fo + nf]
            out[idx[nf:]] = flat[ro : ro + len(idx) - nf]
    return out


def kernel(node_latent, node_types, w1, b1, w2, b2, ln_gamma, ln_beta, head_w, head_b):
    from concourse.bass_utils import run_bass_kernel_spmd

    node_latent = np.asarray(node_latent, dtype=np.float32)
    node_types = np.asarray(node_types)
    blocks, R, regions, idx_by_type = plan(node_types)
    in_maps, use_c1 = prep_inputs(
        node_latent, w1, b1, w2, b2, ln_gamma, ln_beta, head_w, head_b,
        regions, idx_by_type, R,
    )
    nc = build_program(blocks, R, use_c1=use_c1)
    res = run_bass_kernel_spmd(nc, in_maps, core_ids=list(range(N_CORES)))
    return unpack_outputs(res.results, regions, idx_by_type, node_latent.shape[0])
